# revision 3
# baseline (speedup 1.0000x reference)
"""Trainium2 Bass kernel for the fused soft-logic-gate layer.

Reference computation:
    pa = softmax(wa, axis=1); pb = softmax(wb, axis=1); pt = softmax(wt, axis=0)
    A = pa @ x; B = pb @ x
    out = sum_g pt[g,:,None] * gate_g(A, B)        (16 soft logic gates)

Every gate is affine in {1, A, B, A*B}, so the 16-gate table collapses to
    out = c0 + cA*A + cB*B + cAB*(A*B)
with four per-row coefficient vectors derived from pt.  Folding the softmax
denominators of wa/wb into those coefficients lets the matmuls run on the raw
exp() weights, so the device work is: two [256,256]x[256,b] matmuls + one
elementwise pass — memory-bound on streaming x in and out once.

Sharding: batch axis of x split evenly across 8 NeuronCores (data parallel),
weights replicated.  Matmuls run as float32r (TF32) at full PE rate.
"""

import os
import sys

for _p in ("/opt/trn_rl_repo",):
    if _p not in sys.path and os.path.isdir(_p):
        sys.path.insert(0, _p)

import numpy as np

SIZE = 256
PREV = 256
BATCH = 32768
N_CORES = 8
BSH = BATCH // N_CORES  # per-core batch shard
CH = 1024               # epilogue chunk width (2 PSUM banks)
NCH = BSH // CH
P = 128

_CACHE = {}


def _sign_matrix() -> np.ndarray:
    """[16,5] f32: columns are the gate-table coefficients of
    {1, A, B, A*B} plus an all-ones column (softmax denominator)."""
    S = np.zeros((16, 5), dtype=np.float32)
    S[8:16, 0] = 1.0
    for g in (2, 3, 6, 7):
        S[g, 1] += 1.0
    for g in (8, 9, 12, 13):
        S[g, 1] -= 1.0
    for g in (4, 5, 6, 7):
        S[g, 2] += 1.0
    for g in (8, 9, 10, 11):
        S[g, 2] -= 1.0
    for g, v in {1: 1, 2: -1, 4: -1, 6: -2, 7: -1, 8: 1, 9: 2, 11: 1, 13: 1, 14: -1}.items():
        S[g, 3] = v
    S[:, 4] = 1.0
    return S


def _build_bass():
    import concourse.bacc as bacc
    import concourse.tile as tile
    import concourse.mybir as mybir

    f32 = mybir.dt.float32
    f32r = mybir.dt.float32r
    Act = mybir.ActivationFunctionType
    Alu = mybir.AluOpType

    nc = bacc.Bacc(trn_type="TRN2", target_bir_lowering=False, debug=False,
                   num_devices=N_CORES)

    xs_d = nc.dram_tensor("xs", [PREV, BSH], f32r, kind="ExternalInput").ap()
    wa_d = nc.dram_tensor("wa", [SIZE, PREV], f32, kind="ExternalInput").ap()
    wb_d = nc.dram_tensor("wb", [SIZE, PREV], f32, kind="ExternalInput").ap()
    wt_d = nc.dram_tensor("wt", [16, SIZE], f32, kind="ExternalInput").ap()
    id_d = nc.dram_tensor("ident", [P, P], f32, kind="ExternalInput").ap()
    sm_d = nc.dram_tensor("smat", [16, 5], f32, kind="ExternalInput").ap()
    on_d = nc.dram_tensor("onesc", [P, 1], f32, kind="ExternalInput").ap()
    out_d = nc.dram_tensor("out", [SIZE, BSH], f32, kind="ExternalOutput").ap()

    with tile.TileContext(nc) as tc:
        with tc.tile_pool(name="consts", bufs=1) as consts, \
             tc.tile_pool(name="weights", bufs=1) as weights, \
             tc.tile_pool(name="coefs", bufs=1) as coefs:

            ident = consts.tile([P, P], f32)
            nc.sync.dma_start(out=ident[:], in_=id_d[:])
            smat = consts.tile([16, 5], f32)
            nc.sync.dma_start(out=smat[:], in_=sm_d[:])
            onesc = consts.tile([P, 1], f32)
            nc.sync.dma_start(out=onesc[:], in_=on_d[:])

            # Transposed exp(weights), float32r, live for the whole kernel:
            # eaT[p] is [128(prev-block p), 256(size)].
            eaT = [weights.tile([P, SIZE], f32r, tag=f"eaT{p}", name=f"eaT{p}") for p in range(2)]
            ebT = [weights.tile([P, SIZE], f32r, tag=f"ebT{p}", name=f"ebT{p}") for p in range(2)]
            ept = weights.tile([16, SIZE], f32, tag="ept")

            # Per-m-block folded coefficients [128,1]: c0, cA', cB', cAB'.
            c0 = [coefs.tile([P, 1], f32, tag=f"c0_{m}", name=f"c0_{m}") for m in range(2)]
            cA = [coefs.tile([P, 1], f32, tag=f"cA_{m}", name=f"cA_{m}") for m in range(2)]
            cB = [coefs.tile([P, 1], f32, tag=f"cB_{m}", name=f"cB_{m}") for m in range(2)]
            cAB = [coefs.tile([P, 1], f32, tag=f"cAB_{m}", name=f"cAB_{m}") for m in range(2)]

            # ---- weight preprocessing ----
            with tc.tile_pool(name="prep", bufs=2) as prep, \
                 tc.tile_pool(name="prep_ps", bufs=3, space="PSUM") as prep_ps, \
                 tc.tile_pool(name="coef_ps", bufs=2, space="PSUM") as coef_ps:

                recips = {}
                for w_d, eT, nm in ((wa_d, eaT, "a"), (wb_d, ebT, "b")):
                    e_nat = []
                    for m in range(2):
                        wr = prep.tile([P, PREV], f32, tag="wraw")
                        nc.sync.dma_start(out=wr[:], in_=w_d[m * P:(m + 1) * P, :])
                        en = prep.tile([P, PREV], f32, tag=f"e{nm}{m}")
                        nc.scalar.activation(out=en[:], in_=wr[:], func=Act.Exp)
                        e_nat.append(en)
                    # PE transpose each 128x128 block: eT[p][:, mP:(m+1)P] = e_nat[m][:, pP:(p+1)P].T
                    for m in range(2):
                        for p in range(2):
                            tp = prep_ps.tile([P, P], f32, tag="tps")
                            nc.tensor.transpose(tp[:], e_nat[m][:, p * P:(p + 1) * P], ident[:])
                            nc.scalar.copy(out=eT[p][:, m * P:(m + 1) * P], in_=tp[:])
                    # row sums of exp(w) via ones-matmul on the transposed tiles
                    for m in range(2):
                        rs = coef_ps.tile([P, 1], f32, tag="rs")
                        nc.tensor.matmul(rs[:], eaTb(eT, 0, m).bitcast(f32), onesc[:], start=True, stop=False)
                        nc.tensor.matmul(rs[:], eaTb(eT, 1, m).bitcast(f32), onesc[:], start=False, stop=True)
                        rc = coefs.tile([P, 1], f32, tag=f"r{nm}{m}", name=f"r{nm}{m}")
                        nc.vector.reciprocal(out=rc[:], in_=rs[:])
                        recips[(nm, m)] = rc

                wtr = prep.tile([16, SIZE], f32, tag="wtraw")
                nc.sync.dma_start(out=wtr[:], in_=wt_d[:])
                nc.scalar.activation(out=ept[:], in_=wtr[:], func=Act.Exp)

                for m in range(2):
                    cps = coef_ps.tile([P, 5], f32, tag="cps")
                    nc.tensor.matmul(cps[:], ept[:, m * P:(m + 1) * P], smat[:],
                                     start=True, stop=True)
                    rpt = prep.tile([P, 1], f32, tag="rpt")
                    nc.vector.reciprocal(out=rpt[:], in_=cps[:, 4:5])
                    ra, rb = recips[("a", m)], recips[("b", m)]
                    rab = prep.tile([P, 1], f32, tag="rab")
                    nc.vector.tensor_tensor(out=rab[:], in0=ra[:], in1=rb[:], op=Alu.mult)
                    nc.vector.tensor_scalar_mul(c0[m][:], cps[:, 0:1], rpt[:])
                    nc.vector.scalar_tensor_tensor(out=cA[m][:], in0=cps[:, 1:2],
                                                   scalar=rpt[:], in1=ra[:],
                                                   op0=Alu.mult, op1=Alu.mult)
                    nc.vector.scalar_tensor_tensor(out=cB[m][:], in0=cps[:, 2:3],
                                                   scalar=rpt[:], in1=rb[:],
                                                   op0=Alu.mult, op1=Alu.mult)
                    nc.vector.scalar_tensor_tensor(out=cAB[m][:], in0=cps[:, 3:4],
                                                   scalar=rpt[:], in1=rab[:],
                                                   op0=Alu.mult, op1=Alu.mult)

            # ---- main loop ----
            with tc.tile_pool(name="xp", bufs=3) as xp, \
                 tc.tile_pool(name="ep", bufs=3) as ep, \
                 tc.tile_pool(name="mm_ps", bufs=2, space="PSUM") as mm_ps:
                for n in range(NCH):
                    xk = []
                    for k in range(2):
                        xt = xp.tile([P, CH], f32r, tag=f"x{k}")
                        nc.sync.dma_start(out=xt[:], in_=xs_d[k * P:(k + 1) * P, n * CH:(n + 1) * CH])
                        xk.append(xt)
                    for m in range(2):
                        a_ps = mm_ps.tile([P, CH], f32, tag="A")
                        b_ps = mm_ps.tile([P, CH], f32, tag="B")
                        for s in range(CH // 512):
                            sl = slice(s * 512, (s + 1) * 512)
                            nc.tensor.matmul(a_ps[:, sl], eaT[0][:, m * P:(m + 1) * P],
                                             xk[0][:, sl], start=True, stop=False)
                            nc.tensor.matmul(a_ps[:, sl], eaT[1][:, m * P:(m + 1) * P],
                                             xk[1][:, sl], start=False, stop=True)
                        for s in range(CH // 512):
                            sl = slice(s * 512, (s + 1) * 512)
                            nc.tensor.matmul(b_ps[:, sl], ebT[0][:, m * P:(m + 1) * P],
                                             xk[0][:, sl], start=True, stop=False)
                            nc.tensor.matmul(b_ps[:, sl], ebT[1][:, m * P:(m + 1) * P],
                                             xk[1][:, sl], start=False, stop=True)
                        # out = c0 + cA*A + cB*B + cAB*A*B   (A,B raw; coefs folded)
                        s_sb = ep.tile([P, CH], f32, tag="s")
                        nc.scalar.activation(out=s_sb[:], in_=b_ps[:], func=Act.Identity,
                                             scale=cAB[m][:], bias=cA[m][:])
                        p_sb = ep.tile([P, CH], f32, tag="p")
                        nc.vector.tensor_tensor(out=p_sb[:], in0=a_ps[:], in1=s_sb[:],
                                                op=Alu.mult)
                        o_sb = ep.tile([P, CH], f32, tag="o")
                        nc.vector.affine_then_add(out=o_sb[:], in0=b_ps[:], in1=p_sb[:],
                                                  scale=cB[m][:], bias=c0[m][:])
                        nc.sync.dma_start(out=out_d[m * P:(m + 1) * P, n * CH:(n + 1) * CH],
                                          in_=o_sb[:])

    nc.compile()
    return nc


def eaTb(eT, p, m):
    PP = 128
    return eT[p][:, m * PP:(m + 1) * PP]


def _get_nc():
    if "nc" not in _CACHE:
        _CACHE["nc"] = _build_bass()
    return _CACHE["nc"]


def _run(x, wa, wb, wt, trace=False, **spmd_kwargs):
    from concourse import bass_utils

    nc = _get_nc()
    x = np.ascontiguousarray(np.asarray(x, dtype=np.float32))
    wa = np.ascontiguousarray(np.asarray(wa, dtype=np.float32))
    wb = np.ascontiguousarray(np.asarray(wb, dtype=np.float32))
    wt = np.ascontiguousarray(np.asarray(wt, dtype=np.float32))

    ident = np.eye(P, dtype=np.float32)
    smat = _sign_matrix()
    onesc = np.ones((P, 1), dtype=np.float32)

    in_maps = []
    for c in range(N_CORES):
        in_maps.append({
            "xs": np.ascontiguousarray(x[:, c * BSH:(c + 1) * BSH]),
            "wa": wa, "wb": wb, "wt": wt,
            "ident": ident, "smat": smat, "onesc": onesc,
        })
    res = bass_utils.run_bass_kernel_spmd(nc, in_maps, core_ids=list(range(N_CORES)),
                                          trace=trace, **spmd_kwargs)
    out = np.concatenate([res.results[c]["out"] for c in range(N_CORES)], axis=1)
    return out, res


def kernel(x, wa, wb, wt):
    out, _ = _run(x, wa, wb, wt, trace=False)
    return out


# revision 4
# speedup vs baseline: 1.0520x; 1.0520x over previous
"""Trainium2 Bass kernel for the fused soft-logic-gate layer.

Reference computation:
    pa = softmax(wa, axis=1); pb = softmax(wb, axis=1); pt = softmax(wt, axis=0)
    A = pa @ x; B = pb @ x
    out = sum_g pt[g,:,None] * gate_g(A, B)        (16 soft logic gates)

Every gate is affine in {1, A, B, A*B}, so the 16-gate table collapses to
    out = c0 + cA*A + cB*B + cAB*(A*B)
with four per-row coefficient vectors derived from pt.  Folding the softmax
denominators of wa/wb into those coefficients lets the matmuls run on the raw
exp() weights, and factoring
    out = (A + u) * (cAB*B + cA) + w,   u = cB/cAB,  w = c0 - cA*u
leaves one ACT op + two DVE ops per tile.  The device work is two
[256,256]x[256,b] float32r (TF32) matmuls plus that elementwise pass —
memory-bound on streaming x in and out once.

Sharding: batch axis of x split evenly across 8 NeuronCores (data parallel),
weights replicated.
"""

import os
import sys

for _p in ("/opt/trn_rl_repo",):
    if _p not in sys.path and os.path.isdir(_p):
        sys.path.insert(0, _p)

import numpy as np

SIZE = 256
PREV = 256
BATCH = 32768
N_CORES = 8
BSH = BATCH // N_CORES  # per-core batch shard
CH = 1024               # epilogue chunk width (2 PSUM banks)
NCH = BSH // CH
P = 128

_CACHE = {}


def _sign_matrix() -> np.ndarray:
    """[16,5] f32: columns are the gate-table coefficients of
    {1, A, B, A*B} plus an all-ones column (softmax denominator)."""
    S = np.zeros((16, 5), dtype=np.float32)
    S[8:16, 0] = 1.0
    for g in (2, 3, 6, 7):
        S[g, 1] += 1.0
    for g in (8, 9, 12, 13):
        S[g, 1] -= 1.0
    for g in (4, 5, 6, 7):
        S[g, 2] += 1.0
    for g in (8, 9, 10, 11):
        S[g, 2] -= 1.0
    for g, v in {1: 1, 2: -1, 4: -1, 6: -2, 7: -1, 8: 1, 9: 2, 11: 1, 13: 1, 14: -1}.items():
        S[g, 3] = v
    S[:, 4] = 1.0
    return S


def _build_bass():
    import concourse.bacc as bacc
    import concourse.tile as tile
    import concourse.mybir as mybir

    f32 = mybir.dt.float32
    f32r = mybir.dt.float32r
    Act = mybir.ActivationFunctionType
    Alu = mybir.AluOpType

    nc = bacc.Bacc(trn_type="TRN2", target_bir_lowering=False, debug=False,
                   num_devices=N_CORES)

    xs_d = nc.dram_tensor("xs", [PREV, BSH], f32r, kind="ExternalInput").ap()
    wa_d = nc.dram_tensor("wa", [SIZE, PREV], f32, kind="ExternalInput").ap()
    wb_d = nc.dram_tensor("wb", [SIZE, PREV], f32, kind="ExternalInput").ap()
    wt_d = nc.dram_tensor("wt", [16, SIZE], f32, kind="ExternalInput").ap()
    id_d = nc.dram_tensor("ident", [P, P], f32, kind="ExternalInput").ap()
    sm_d = nc.dram_tensor("smat", [16, 5], f32, kind="ExternalInput").ap()
    on_d = nc.dram_tensor("onesc", [P, 1], f32, kind="ExternalInput").ap()
    out_d = nc.dram_tensor("out", [SIZE, BSH], f32, kind="ExternalOutput").ap()

    # [p, k, b] / [p, m, b] views of the DRAM tensors for single-DMA transfers
    xs_v = xs_d.rearrange("(k p) b -> p k b", p=P)
    out_v = out_d.rearrange("(m p) b -> p m b", p=P)

    with tile.TileContext(nc) as tc:
        with tc.tile_pool(name="consts", bufs=1) as consts, \
             tc.tile_pool(name="weights", bufs=1) as weights, \
             tc.tile_pool(name="coefs", bufs=1) as coefs, \
             tc.tile_pool(name="xp", bufs=4) as xp:

            # prefetch the first x chunks before anything else
            xtiles = []
            for n in range(2):
                xt = xp.tile([P, 2, CH], f32r, tag="x", name=f"x{n}")
                nc.sync.dma_start(out=xt[:], in_=xs_v[:, :, n * CH:(n + 1) * CH])
                xtiles.append(xt)

            ident = consts.tile([P, P], f32)
            nc.gpsimd.dma_start(out=ident[:], in_=id_d[:])
            smat = consts.tile([16, 5], f32)
            nc.gpsimd.dma_start(out=smat[:], in_=sm_d[:])
            onesc = consts.tile([P, 1], f32)
            nc.gpsimd.dma_start(out=onesc[:], in_=on_d[:])

            # Transposed exp(weights), float32r, live for the whole kernel:
            # eaT[p] is [128(prev-block p), 256(size)].
            eaT = [weights.tile([P, SIZE], f32r, tag=f"eaT{p}", name=f"eaT{p}") for p in range(2)]
            ebT = [weights.tile([P, SIZE], f32r, tag=f"ebT{p}", name=f"ebT{p}") for p in range(2)]
            ept = weights.tile([16, SIZE], f32, tag="ept")

            # Per-m-block coefficient scalars [128,1] for the epilogue:
            # out = (A + u) * (cAB*B + cA) + w
            cA = [coefs.tile([P, 1], f32, tag=f"cA_{m}", name=f"cA_{m}") for m in range(2)]
            cAB = [coefs.tile([P, 1], f32, tag=f"cAB_{m}", name=f"cAB_{m}") for m in range(2)]
            cU = [coefs.tile([P, 1], f32, tag=f"cU_{m}", name=f"cU_{m}") for m in range(2)]
            cW = [coefs.tile([P, 1], f32, tag=f"cW_{m}", name=f"cW_{m}") for m in range(2)]

            # ---- weight preprocessing ----
            with tc.tile_pool(name="prep", bufs=2) as prep, \
                 tc.tile_pool(name="prep_ps", bufs=3, space="PSUM") as prep_ps, \
                 tc.tile_pool(name="coef_ps", bufs=2, space="PSUM") as coef_ps:

                recips = {}
                for w_d, eT, nm in ((wa_d, eaT, "a"), (wb_d, ebT, "b")):
                    e_nat = []
                    for m in range(2):
                        wr = prep.tile([P, PREV], f32, tag="wraw", name=f"wr{nm}{m}")
                        nc.gpsimd.dma_start(out=wr[:], in_=w_d[m * P:(m + 1) * P, :])
                        en = prep.tile([P, PREV], f32, tag=f"e{nm}{m}", name=f"e{nm}{m}")
                        nc.scalar.activation(out=en[:], in_=wr[:], func=Act.Exp)
                        e_nat.append(en)
                    # PE transpose each 128x128 block: eT[p][:, mP:(m+1)P] = e_nat[m][:, pP:(p+1)P].T
                    for m in range(2):
                        for p in range(2):
                            tp = prep_ps.tile([P, P], f32, tag="tps", name=f"tp{nm}{m}{p}")
                            nc.tensor.transpose(tp[:], e_nat[m][:, p * P:(p + 1) * P], ident[:])
                            nc.scalar.copy(out=eT[p][:, m * P:(m + 1) * P], in_=tp[:])
                    # row sums of exp(w) via ones-matmul on the transposed tiles
                    for m in range(2):
                        rs = coef_ps.tile([P, 1], f32, tag="rs", name=f"rs{nm}{m}")
                        nc.tensor.matmul(rs[:], eT[0][:, m * P:(m + 1) * P].bitcast(f32),
                                         onesc[:], start=True, stop=False)
                        nc.tensor.matmul(rs[:], eT[1][:, m * P:(m + 1) * P].bitcast(f32),
                                         onesc[:], start=False, stop=True)
                        rc = coefs.tile([P, 1], f32, tag=f"r{nm}{m}", name=f"r{nm}{m}")
                        nc.vector.reciprocal(out=rc[:], in_=rs[:])
                        recips[(nm, m)] = rc

                wtr = prep.tile([16, SIZE], f32, tag="wtraw")
                nc.gpsimd.dma_start(out=wtr[:], in_=wt_d[:])
                nc.scalar.activation(out=ept[:], in_=wtr[:], func=Act.Exp)

                for m in range(2):
                    cps = coef_ps.tile([P, 5], f32, tag="cps", name=f"cps{m}")
                    nc.tensor.matmul(cps[:], ept[:, m * P:(m + 1) * P], smat[:],
                                     start=True, stop=True)
                    rpt = prep.tile([P, 1], f32, tag="rpt", name=f"rpt{m}")
                    nc.vector.reciprocal(out=rpt[:], in_=cps[:, 4:5])
                    ra, rb = recips[("a", m)], recips[("b", m)]
                    rab = prep.tile([P, 1], f32, tag="rab", name=f"rab{m}")
                    nc.vector.tensor_tensor(out=rab[:], in0=ra[:], in1=rb[:], op=Alu.mult)
                    # folded coefficients
                    c0m = prep.tile([P, 1], f32, tag="c0m", name=f"c0m{m}")
                    nc.vector.tensor_scalar_mul(c0m[:], cps[:, 0:1], rpt[:])
                    nc.vector.scalar_tensor_tensor(out=cA[m][:], in0=cps[:, 1:2],
                                                   scalar=rpt[:], in1=ra[:],
                                                   op0=Alu.mult, op1=Alu.mult)
                    cBm = prep.tile([P, 1], f32, tag="cBm", name=f"cBm{m}")
                    nc.vector.scalar_tensor_tensor(out=cBm[:], in0=cps[:, 2:3],
                                                   scalar=rpt[:], in1=rb[:],
                                                   op0=Alu.mult, op1=Alu.mult)
                    nc.vector.scalar_tensor_tensor(out=cAB[m][:], in0=cps[:, 3:4],
                                                   scalar=rpt[:], in1=rab[:],
                                                   op0=Alu.mult, op1=Alu.mult)
                    # u = cB / cAB ; w = c0 - cA*u
                    rcab = prep.tile([P, 1], f32, tag="rcab", name=f"rcab{m}")
                    nc.vector.reciprocal(out=rcab[:], in_=cAB[m][:])
                    nc.vector.tensor_tensor(out=cU[m][:], in0=cBm[:], in1=rcab[:], op=Alu.mult)
                    negw = prep.tile([P, 1], f32, tag="negw", name=f"negw{m}")
                    nc.vector.scalar_tensor_tensor(out=negw[:], in0=cA[m][:],
                                                   scalar=cU[m][:], in1=c0m[:],
                                                   op0=Alu.mult, op1=Alu.subtract)
                    nc.vector.tensor_scalar_mul(cW[m][:], negw[:], -1.0)

            # ---- main loop ----
            with tc.tile_pool(name="ep", bufs=3) as ep, \
                 tc.tile_pool(name="mm_ps", bufs=2, space="PSUM") as mm_ps:
                for n in range(NCH):
                    if n + 2 < NCH:
                        xt = xp.tile([P, 2, CH], f32r, tag="x", name=f"x{n+2}")
                        nc.sync.dma_start(out=xt[:], in_=xs_v[:, :, (n + 2) * CH:(n + 3) * CH])
                        xtiles.append(xt)
                    xk = xtiles[n]
                    o_sb = ep.tile([P, 2, CH], f32, tag="o", name=f"o{n}")
                    for m in range(2):
                        a_ps = mm_ps.tile([P, CH], f32, tag="A", name=f"A{n}{m}")
                        b_ps = mm_ps.tile([P, CH], f32, tag="B", name=f"B{n}{m}")
                        for ps_t, eT in ((a_ps, eaT), (b_ps, ebT)):
                            for k in range(2):
                                for s in range(CH // 512):
                                    sl = slice(s * 512, (s + 1) * 512)
                                    nc.tensor.matmul(ps_t[:, sl],
                                                     eT[k][:, m * P:(m + 1) * P],
                                                     xk[:, k, sl],
                                                     start=(k == 0), stop=(k == 1))
                        # out = (A + u) * (cAB*B + cA) + w
                        s_sb = ep.tile([P, CH], f32, tag="s", name=f"s{n}{m}")
                        nc.scalar.activation(out=s_sb[:], in_=b_ps[:], func=Act.Identity,
                                             scale=cAB[m][:], bias=cA[m][:])
                        p_sb = ep.tile([P, CH], f32, tag="p", name=f"p{n}{m}")
                        nc.vector.scalar_tensor_tensor(out=p_sb[:], in0=a_ps[:],
                                                       scalar=cU[m][:], in1=s_sb[:],
                                                       op0=Alu.add, op1=Alu.mult)
                        nc.vector.tensor_scalar_add(o_sb[:, m, :], p_sb[:], cW[m][:])
                    nc.sync.dma_start(out=out_v[:, :, n * CH:(n + 1) * CH], in_=o_sb[:])

    nc.compile()
    return nc


def _get_nc():
    if "nc" not in _CACHE:
        _CACHE["nc"] = _build_bass()
    return _CACHE["nc"]


def _run(x, wa, wb, wt, trace=False, **spmd_kwargs):
    from concourse import bass_utils

    nc = _get_nc()
    x = np.ascontiguousarray(np.asarray(x, dtype=np.float32))
    wa = np.ascontiguousarray(np.asarray(wa, dtype=np.float32))
    wb = np.ascontiguousarray(np.asarray(wb, dtype=np.float32))
    wt = np.ascontiguousarray(np.asarray(wt, dtype=np.float32))

    ident = np.eye(P, dtype=np.float32)
    smat = _sign_matrix()
    onesc = np.ones((P, 1), dtype=np.float32)

    in_maps = []
    for c in range(N_CORES):
        in_maps.append({
            "xs": np.ascontiguousarray(x[:, c * BSH:(c + 1) * BSH]),
            "wa": wa, "wb": wb, "wt": wt,
            "ident": ident, "smat": smat, "onesc": onesc,
        })
    res = bass_utils.run_bass_kernel_spmd(nc, in_maps, core_ids=list(range(N_CORES)),
                                          trace=trace, **spmd_kwargs)
    out = np.concatenate([res.results[c]["out"] for c in range(N_CORES)], axis=1)
    return out, res


def kernel(x, wa, wb, wt):
    out, _ = _run(x, wa, wb, wt, trace=False)
    return out


# revision 5
# speedup vs baseline: 1.3045x; 1.2400x over previous
"""Trainium2 Bass kernel for the fused soft-logic-gate layer.

Reference computation:
    pa = softmax(wa, axis=1); pb = softmax(wb, axis=1); pt = softmax(wt, axis=0)
    A = pa @ x; B = pb @ x
    out = sum_g pt[g,:,None] * gate_g(A, B)        (16 soft logic gates)

Every gate is affine in {1, A, B, A*B}, so the 16-gate table collapses to
    out = c0 + cA*A + cB*B + cAB*(A*B)
with four per-row coefficient vectors derived from pt.  Folding the softmax
denominators of wa/wb into those coefficients lets the matmuls run on the raw
exp() weights, and factoring
    out = (A + u) * (cAB*B + cA) + w,   u = cB/cAB,  w = c0 - cA*u
leaves one ACT op + two DVE ops per tile.  The device work is two
[256,256]x[256,b] float32r (TF32) matmuls plus that elementwise pass —
memory-bound on streaming x in and out once.

Sharding: batch axis of x split evenly across 8 NeuronCores (data parallel),
weights replicated.
"""

import os
import sys

for _p in ("/opt/trn_rl_repo",):
    if _p not in sys.path and os.path.isdir(_p):
        sys.path.insert(0, _p)

import numpy as np

SIZE = 256
PREV = 256
BATCH = 32768
N_CORES = 8
BSH = BATCH // N_CORES  # per-core batch shard
CH = 1024               # epilogue chunk width (2 PSUM banks)
NCH = BSH // CH
P = 128

_CACHE = {}


def _sign_matrix() -> np.ndarray:
    """[16,5] f32: columns are the gate-table coefficients of
    {1, A, B, A*B} plus an all-ones column (softmax denominator)."""
    S = np.zeros((16, 5), dtype=np.float32)
    S[8:16, 0] = 1.0
    for g in (2, 3, 6, 7):
        S[g, 1] += 1.0
    for g in (8, 9, 12, 13):
        S[g, 1] -= 1.0
    for g in (4, 5, 6, 7):
        S[g, 2] += 1.0
    for g in (8, 9, 10, 11):
        S[g, 2] -= 1.0
    for g, v in {1: 1, 2: -1, 4: -1, 6: -2, 7: -1, 8: 1, 9: 2, 11: 1, 13: 1, 14: -1}.items():
        S[g, 3] = v
    S[:, 4] = 1.0
    return S


def _build_bass():
    import concourse.bacc as bacc
    import concourse.tile as tile
    import concourse.mybir as mybir

    f32 = mybir.dt.float32
    f32r = mybir.dt.float32r
    Act = mybir.ActivationFunctionType
    Alu = mybir.AluOpType

    nc = bacc.Bacc(trn_type="TRN2", target_bir_lowering=False, debug=False,
                   num_devices=N_CORES)

    xs_d = nc.dram_tensor("xs", [PREV, BSH], f32r, kind="ExternalInput").ap()
    wa_d = nc.dram_tensor("wa", [SIZE, PREV], f32, kind="ExternalInput").ap()
    wb_d = nc.dram_tensor("wb", [SIZE, PREV], f32, kind="ExternalInput").ap()
    wt_d = nc.dram_tensor("wt", [16, SIZE], f32, kind="ExternalInput").ap()
    id_d = nc.dram_tensor("ident", [P, P], f32, kind="ExternalInput").ap()
    sm_d = nc.dram_tensor("smat", [16, 5], f32, kind="ExternalInput").ap()
    on_d = nc.dram_tensor("onesc", [P, 1], f32, kind="ExternalInput").ap()
    out_d = nc.dram_tensor("out", [SIZE, BSH], f32, kind="ExternalOutput").ap()

    # [p, k, b] / [p, m, b] views of the DRAM tensors for single-DMA transfers
    xs_v = xs_d.rearrange("(k p) b -> p k b", p=P)
    out_v = out_d.rearrange("(m p) b -> p m b", p=P)

    with tile.TileContext(nc) as tc:
        with tc.tile_pool(name="consts", bufs=1) as consts, \
             tc.tile_pool(name="weights", bufs=1) as weights, \
             tc.tile_pool(name="coefs", bufs=1) as coefs, \
             tc.tile_pool(name="xp", bufs=4) as xp:

            # constants + weights first: they gate the whole compute chain
            ident = consts.tile([P, P], f32)
            nc.sync.dma_start(out=ident[:], in_=id_d[:])
            wab_sb = []
            for w_d, nm in ((wa_d, "a"), (wb_d, "b")):
                for m in range(2):
                    wr = consts.tile([P, PREV], f32, tag=f"wr{nm}{m}", name=f"wr{nm}{m}")
                    nc.sync.dma_start(out=wr[:], in_=w_d[m * P:(m + 1) * P, :])
                    wab_sb.append(wr)
            wtr = consts.tile([16, SIZE], f32, tag="wtraw")
            nc.sync.dma_start(out=wtr[:], in_=wt_d[:])
            smat = consts.tile([16, 5], f32)
            nc.sync.dma_start(out=smat[:], in_=sm_d[:])
            onesc = consts.tile([P, 1], f32)
            nc.sync.dma_start(out=onesc[:], in_=on_d[:])

            # prefetch the first x chunks
            xtiles = []
            for n in range(2):
                xt = xp.tile([P, 2, CH], f32r, tag="x", name=f"x{n}")
                nc.sync.dma_start(out=xt[:], in_=xs_v[:, :, n * CH:(n + 1) * CH])
                xtiles.append(xt)

            # Transposed exp(weights), float32r, live for the whole kernel:
            # eaT[p] is [128(prev-block p), 256(size)].
            eaT = [weights.tile([P, SIZE], f32r, tag=f"eaT{p}", name=f"eaT{p}") for p in range(2)]
            ebT = [weights.tile([P, SIZE], f32r, tag=f"ebT{p}", name=f"ebT{p}") for p in range(2)]
            ept = weights.tile([16, SIZE], f32, tag="ept")

            # Per-m-block coefficient scalars [128,1] for the epilogue:
            # out = (A + u) * (cAB*B + cA) + w
            cA = [coefs.tile([P, 1], f32, tag=f"cA_{m}", name=f"cA_{m}") for m in range(2)]
            cAB = [coefs.tile([P, 1], f32, tag=f"cAB_{m}", name=f"cAB_{m}") for m in range(2)]
            cU = [coefs.tile([P, 1], f32, tag=f"cU_{m}", name=f"cU_{m}") for m in range(2)]
            cW = [coefs.tile([P, 1], f32, tag=f"cW_{m}", name=f"cW_{m}") for m in range(2)]

            # ---- weight preprocessing ----
            with tc.tile_pool(name="prep", bufs=2) as prep, \
                 tc.tile_pool(name="prep_ps", bufs=3, space="PSUM") as prep_ps, \
                 tc.tile_pool(name="coef_ps", bufs=2, space="PSUM") as coef_ps:

                recips = {}
                for wi, (eT, nm) in enumerate(((eaT, "a"), (ebT, "b"))):
                    e_nat = []
                    for m in range(2):
                        wr = wab_sb[wi * 2 + m]
                        en = prep.tile([P, PREV], f32, tag=f"e{nm}{m}", name=f"e{nm}{m}")
                        nc.scalar.activation(out=en[:], in_=wr[:], func=Act.Exp)
                        e_nat.append(en)
                    # PE transpose each 128x128 block: eT[p][:, mP:(m+1)P] = e_nat[m][:, pP:(p+1)P].T
                    for m in range(2):
                        for p in range(2):
                            tp = prep_ps.tile([P, P], f32, tag="tps", name=f"tp{nm}{m}{p}")
                            nc.tensor.transpose(tp[:], e_nat[m][:, p * P:(p + 1) * P], ident[:])
                            nc.scalar.copy(out=eT[p][:, m * P:(m + 1) * P], in_=tp[:])
                    # row sums of exp(w) via ones-matmul on the transposed tiles
                    for m in range(2):
                        rs = coef_ps.tile([P, 1], f32, tag="rs", name=f"rs{nm}{m}")
                        nc.tensor.matmul(rs[:], eT[0][:, m * P:(m + 1) * P].bitcast(f32),
                                         onesc[:], start=True, stop=False)
                        nc.tensor.matmul(rs[:], eT[1][:, m * P:(m + 1) * P].bitcast(f32),
                                         onesc[:], start=False, stop=True)
                        rc = coefs.tile([P, 1], f32, tag=f"r{nm}{m}", name=f"r{nm}{m}")
                        nc.vector.reciprocal(out=rc[:], in_=rs[:])
                        recips[(nm, m)] = rc

                nc.scalar.activation(out=ept[:], in_=wtr[:], func=Act.Exp)

                for m in range(2):
                    cps = coef_ps.tile([P, 5], f32, tag="cps", name=f"cps{m}")
                    nc.tensor.matmul(cps[:], ept[:, m * P:(m + 1) * P], smat[:],
                                     start=True, stop=True)
                    rpt = prep.tile([P, 1], f32, tag="rpt", name=f"rpt{m}")
                    nc.vector.reciprocal(out=rpt[:], in_=cps[:, 4:5])
                    ra, rb = recips[("a", m)], recips[("b", m)]
                    rab = prep.tile([P, 1], f32, tag="rab", name=f"rab{m}")
                    nc.vector.tensor_tensor(out=rab[:], in0=ra[:], in1=rb[:], op=Alu.mult)
                    # folded coefficients
                    c0m = prep.tile([P, 1], f32, tag="c0m", name=f"c0m{m}")
                    nc.vector.tensor_scalar_mul(c0m[:], cps[:, 0:1], rpt[:])
                    nc.vector.scalar_tensor_tensor(out=cA[m][:], in0=cps[:, 1:2],
                                                   scalar=rpt[:], in1=ra[:],
                                                   op0=Alu.mult, op1=Alu.mult)
                    cBm = prep.tile([P, 1], f32, tag="cBm", name=f"cBm{m}")
                    nc.vector.scalar_tensor_tensor(out=cBm[:], in0=cps[:, 2:3],
                                                   scalar=rpt[:], in1=rb[:],
                                                   op0=Alu.mult, op1=Alu.mult)
                    nc.vector.scalar_tensor_tensor(out=cAB[m][:], in0=cps[:, 3:4],
                                                   scalar=rpt[:], in1=rab[:],
                                                   op0=Alu.mult, op1=Alu.mult)
                    # u = cB / cAB ; w = c0 - cA*u
                    rcab = prep.tile([P, 1], f32, tag="rcab", name=f"rcab{m}")
                    nc.vector.reciprocal(out=rcab[:], in_=cAB[m][:])
                    nc.vector.tensor_tensor(out=cU[m][:], in0=cBm[:], in1=rcab[:], op=Alu.mult)
                    negw = prep.tile([P, 1], f32, tag="negw", name=f"negw{m}")
                    nc.vector.scalar_tensor_tensor(out=negw[:], in0=cA[m][:],
                                                   scalar=cU[m][:], in1=c0m[:],
                                                   op0=Alu.mult, op1=Alu.subtract)
                    nc.vector.tensor_scalar_mul(cW[m][:], negw[:], -1.0)

            # ---- main loop ----
            with tc.tile_pool(name="ep", bufs=3) as ep, \
                 tc.tile_pool(name="mm_ps", bufs=2, space="PSUM") as mm_ps:
                for n in range(NCH):
                    if n + 2 < NCH:
                        xt = xp.tile([P, 2, CH], f32r, tag="x", name=f"x{n+2}")
                        nc.sync.dma_start(out=xt[:], in_=xs_v[:, :, (n + 2) * CH:(n + 3) * CH])
                        xtiles.append(xt)
                    xk = xtiles[n]
                    o_sb = ep.tile([P, 2, CH], f32, tag="o", name=f"o{n}")
                    for m in range(2):
                        a_ps = mm_ps.tile([P, CH], f32, tag="A", name=f"A{n}{m}")
                        b_ps = mm_ps.tile([P, CH], f32, tag="B", name=f"B{n}{m}")
                        for ps_t, eT in ((a_ps, eaT), (b_ps, ebT)):
                            for k in range(2):
                                for s in range(CH // 512):
                                    sl = slice(s * 512, (s + 1) * 512)
                                    nc.tensor.matmul(ps_t[:, sl],
                                                     eT[k][:, m * P:(m + 1) * P],
                                                     xk[:, k, sl],
                                                     start=(k == 0), stop=(k == 1))
                        # out = (A + u) * (cAB*B + cA) + w
                        s_sb = ep.tile([P, CH], f32, tag="s", name=f"s{n}{m}")
                        nc.scalar.activation(out=s_sb[:], in_=b_ps[:], func=Act.Identity,
                                             scale=cAB[m][:], bias=cA[m][:])
                        p_sb = ep.tile([P, CH], f32, tag="p", name=f"p{n}{m}")
                        nc.vector.scalar_tensor_tensor(out=p_sb[:], in0=a_ps[:],
                                                       scalar=cU[m][:], in1=s_sb[:],
                                                       op0=Alu.add, op1=Alu.mult)
                        nc.vector.tensor_scalar_add(o_sb[:, m, :], p_sb[:], cW[m][:])
                    nc.sync.dma_start(out=out_v[:, :, n * CH:(n + 1) * CH], in_=o_sb[:])

    nc.compile()
    return nc


def _get_nc():
    if "nc" not in _CACHE:
        _CACHE["nc"] = _build_bass()
    return _CACHE["nc"]


def _run(x, wa, wb, wt, trace=False, **spmd_kwargs):
    from concourse import bass_utils

    nc = _get_nc()
    x = np.ascontiguousarray(np.asarray(x, dtype=np.float32))
    wa = np.ascontiguousarray(np.asarray(wa, dtype=np.float32))
    wb = np.ascontiguousarray(np.asarray(wb, dtype=np.float32))
    wt = np.ascontiguousarray(np.asarray(wt, dtype=np.float32))

    ident = np.eye(P, dtype=np.float32)
    smat = _sign_matrix()
    onesc = np.ones((P, 1), dtype=np.float32)

    in_maps = []
    for c in range(N_CORES):
        in_maps.append({
            "xs": np.ascontiguousarray(x[:, c * BSH:(c + 1) * BSH]),
            "wa": wa, "wb": wb, "wt": wt,
            "ident": ident, "smat": smat, "onesc": onesc,
        })
    res = bass_utils.run_bass_kernel_spmd(nc, in_maps, core_ids=list(range(N_CORES)),
                                          trace=trace, **spmd_kwargs)
    out = np.concatenate([res.results[c]["out"] for c in range(N_CORES)], axis=1)
    return out, res


def kernel(x, wa, wb, wt):
    out, _ = _run(x, wa, wb, wt, trace=False)
    return out


# revision 7
# speedup vs baseline: 1.3229x; 1.0141x over previous
"""Trainium2 Bass kernel for the fused soft-logic-gate layer.

Reference computation:
    pa = softmax(wa, axis=1); pb = softmax(wb, axis=1); pt = softmax(wt, axis=0)
    A = pa @ x; B = pb @ x
    out = sum_g pt[g,:,None] * gate_g(A, B)        (16 soft logic gates)

Every gate is affine in {1, A, B, A*B}, so the 16-gate table collapses to
    out = c0 + cA*A + cB*B + cAB*(A*B)
with four per-row coefficient vectors derived from pt.  Folding the softmax
denominators of wa/wb into those coefficients lets the matmuls run on the raw
exp() weights, and factoring
    out = (A + u) * (cAB*B + cA) + w,   u = cB/cAB,  w = c0 - cA*u
leaves one ACT op + two DVE ops per tile.  The device work is two
[256,256]x[256,b] float32r (TF32) matmuls plus that elementwise pass —
memory-bound on streaming x in and out once.

Sharding: batch axis of x split evenly across 8 NeuronCores (data parallel),
weights replicated.
"""

import os
import sys

for _p in ("/opt/trn_rl_repo",):
    if _p not in sys.path and os.path.isdir(_p):
        sys.path.insert(0, _p)

import numpy as np

SIZE = 256
PREV = 256
BATCH = 32768
N_CORES = 8
BSH = BATCH // N_CORES  # per-core batch shard
CH = 1024               # epilogue chunk width (2 PSUM banks)
NCH = BSH // CH
P = 128

# constants blob layout (f32, [128, 390]):
#   [:, 0:128]    identity
#   [:, 128]      ones column
#   [:16, 129:134] sign matrix [16, 5] (cols: sum, c0, cA, cB, cAB)
#   [:16, 134:390] wt [16, 256]
BLOB_W = 390

_CACHE = {}


def _sign_matrix() -> np.ndarray:
    """[16,5] f32 columns: [colsum, c0, cA, cB, cAB] — gate-table
    coefficients of {1, A, B, A*B} preceded by the softmax denominator."""
    S = np.zeros((16, 5), dtype=np.float32)
    S[:, 0] = 1.0
    S[8:16, 1] = 1.0
    for g in (2, 3, 6, 7):
        S[g, 2] += 1.0
    for g in (8, 9, 12, 13):
        S[g, 2] -= 1.0
    for g in (4, 5, 6, 7):
        S[g, 3] += 1.0
    for g in (8, 9, 10, 11):
        S[g, 3] -= 1.0
    for g, v in {1: 1, 2: -1, 4: -1, 6: -2, 7: -1, 8: 1, 9: 2, 11: 1, 13: 1, 14: -1}.items():
        S[g, 4] = v
    return S


def _build_bass():
    import concourse.bacc as bacc
    import concourse.tile as tile
    import concourse.mybir as mybir

    f32 = mybir.dt.float32
    f32r = mybir.dt.float32r
    Act = mybir.ActivationFunctionType
    Alu = mybir.AluOpType

    nc = bacc.Bacc(trn_type="TRN2", target_bir_lowering=False, debug=False,
                   num_devices=N_CORES)

    xs_d = nc.dram_tensor("xs", [PREV, BSH], f32r, kind="ExternalInput").ap()
    wa_d = nc.dram_tensor("wa", [SIZE, PREV], f32, kind="ExternalInput").ap()
    wb_d = nc.dram_tensor("wb", [SIZE, PREV], f32, kind="ExternalInput").ap()
    bl_d = nc.dram_tensor("blob", [P, BLOB_W], f32, kind="ExternalInput").ap()
    out_d = nc.dram_tensor("out", [SIZE, BSH], f32, kind="ExternalOutput").ap()

    # [p, k, b] views for single-DMA transfers
    xs_v = xs_d.rearrange("(k p) b -> p k b", p=P)
    wa_v = wa_d.rearrange("(m p) c -> p m c", p=P)
    wb_v = wb_d.rearrange("(m p) c -> p m c", p=P)

    with tile.TileContext(nc) as tc:
        with tc.tile_pool(name="consts", bufs=1) as consts, \
             tc.tile_pool(name="weights", bufs=1) as weights, \
             tc.tile_pool(name="coefs", bufs=1) as coefs, \
             tc.tile_pool(name="xp", bufs=4) as xp:

            blob = consts.tile([P, BLOB_W], f32)
            nc.sync.dma_start(out=blob[:], in_=bl_d[:])
            ident = blob[:, 0:128]
            onesc = blob[:, 128:129]
            smat = blob[:16, 129:134]
            wts = blob[:16, 134:390]

            # tiny early Exp forces the ACT table load off the critical path
            dummy = consts.tile([1, 1], f32)
            nc.scalar.activation(out=dummy[:], in_=blob[0:1, 128:129], func=Act.Exp)

            wa_sb = consts.tile([P, 2, PREV], f32)
            nc.sync.dma_start(out=wa_sb[:], in_=wa_v[:])
            wb_sb = consts.tile([P, 2, PREV], f32)
            nc.sync.dma_start(out=wb_sb[:], in_=wb_v[:])

            # prefetch all x chunks
            xtiles = []
            for n in range(NCH):
                xt = xp.tile([P, 2, CH], f32r, tag="x", name=f"x{n}")
                nc.sync.dma_start(out=xt[:], in_=xs_v[:, :, n * CH:(n + 1) * CH])
                xtiles.append(xt)

            # Transposed exp(weights), float32r, live for the whole kernel:
            # eaT[p] is [128(prev-block p), 256(size)].
            eaT = [weights.tile([P, SIZE], f32r, tag=f"eaT{p}", name=f"eaT{p}") for p in range(2)]
            ebT = [weights.tile([P, SIZE], f32r, tag=f"ebT{p}", name=f"ebT{p}") for p in range(2)]

            # [128,2] coefficient tiles (m as free dim):
            cA2 = coefs.tile([P, 2], f32, tag="cA2")
            cAB2 = coefs.tile([P, 2], f32, tag="cAB2")
            cU2 = coefs.tile([P, 2], f32, tag="cU2")
            cW2 = coefs.tile([P, 2], f32, tag="cW2")

            # ---- weight preprocessing ----
            with tc.tile_pool(name="prep", bufs=2) as prep, \
                 tc.tile_pool(name="prep_ps", bufs=3, space="PSUM") as prep_ps, \
                 tc.tile_pool(name="coef_ps", bufs=1, space="PSUM") as coef_ps:

                # exp in natural layout (one wide ACT op per weight), then
                # PE-transpose each 128x128 block and copy out as float32r
                for w_sb, eT, nm in ((wa_sb, eaT, "a"), (wb_sb, ebT, "b")):
                    e_nat = prep.tile([P, 2, PREV], f32, tag=f"e{nm}", name=f"e{nm}")
                    nc.scalar.activation(out=e_nat[:], in_=w_sb[:], func=Act.Exp)
                    for m in range(2):
                        for p in range(2):
                            tp = prep_ps.tile([P, P], f32, tag="tps", name=f"tp{nm}{m}{p}")
                            nc.tensor.transpose(tp[:], e_nat[:, m, p * P:(p + 1) * P], ident)
                            nc.scalar.copy(out=eT[p][:, m * P:(m + 1) * P], in_=tp[:])

                ept = prep.tile([16, SIZE], f32, tag="ept")
                nc.scalar.activation(out=ept[:], in_=wts, func=Act.Exp)

                # pt-coefficient matmuls: cps[:, m*5:(m+1)*5] = ept[:, mP:(m+1)P].T @ S
                cps = coef_ps.tile([P, 10], f32, tag="cps")
                for m in range(2):
                    nc.tensor.matmul(cps[:, m * 5:(m + 1) * 5],
                                     ept[:, m * P:(m + 1) * P], smat,
                                     start=True, stop=True)
                cpsv = cps[:].rearrange("p (m c) -> p c m", m=2)

                # row sums of exp(w) via ones-matmul on the transposed tiles
                rsa = coef_ps.tile([P, 2], f32, tag="rsa")
                rsb = coef_ps.tile([P, 2], f32, tag="rsb")
                for eT, rs in ((eaT, rsa), (ebT, rsb)):
                    for m in range(2):
                        nc.tensor.matmul(rs[:, m:m + 1], eT[0][:, m * P:(m + 1) * P].bitcast(f32),
                                         onesc, start=True, stop=False)
                        nc.tensor.matmul(rs[:, m:m + 1], eT[1][:, m * P:(m + 1) * P].bitcast(f32),
                                         onesc, start=False, stop=True)

                # batched [128,2] coefficient chain:
                rpt2 = prep.tile([P, 2], f32, tag="rpt2")
                nc.vector.reciprocal(out=rpt2[:], in_=cpsv[:, 0, :])
                ra2 = prep.tile([P, 2], f32, tag="ra2")
                nc.vector.reciprocal(out=ra2[:], in_=rsa[:])
                rb2 = prep.tile([P, 2], f32, tag="rb2")
                nc.vector.reciprocal(out=rb2[:], in_=rsb[:])
                rcabn = prep.tile([P, 2], f32, tag="rcabn")
                nc.vector.reciprocal(out=rcabn[:], in_=cpsv[:, 4, :])

                h2 = prep.tile([P, 2], f32, tag="h2")
                nc.vector.tensor_tensor(out=h2[:], in0=rpt2[:], in1=ra2[:], op=Alu.mult)
                nc.vector.tensor_tensor(out=cA2[:], in0=cpsv[:, 2, :], in1=h2[:], op=Alu.mult)
                g2 = prep.tile([P, 2], f32, tag="g2")
                nc.vector.tensor_tensor(out=g2[:], in0=h2[:], in1=rb2[:], op=Alu.mult)
                nc.vector.tensor_tensor(out=cAB2[:], in0=cpsv[:, 4, :], in1=g2[:], op=Alu.mult)

                # u = cBn * sa / cABn ;  w = c0n*rpt - cA*u
                u2a = prep.tile([P, 2], f32, tag="u2a")
                nc.vector.tensor_tensor(out=u2a[:], in0=cpsv[:, 3, :], in1=rcabn[:], op=Alu.mult)
                nc.vector.tensor_tensor(out=cU2[:], in0=u2a[:], in1=rsa[:], op=Alu.mult)
                c02 = prep.tile([P, 2], f32, tag="c02")
                nc.vector.tensor_tensor(out=c02[:], in0=cpsv[:, 1, :], in1=rpt2[:], op=Alu.mult)
                t2 = prep.tile([P, 2], f32, tag="t2")
                nc.vector.tensor_tensor(out=t2[:], in0=cA2[:], in1=cU2[:], op=Alu.mult)
                nc.vector.tensor_tensor(out=cW2[:], in0=c02[:], in1=t2[:], op=Alu.subtract)

            # ---- main loop ----
            with tc.tile_pool(name="ep", bufs=3) as ep, \
                 tc.tile_pool(name="mm_ps", bufs=2, space="PSUM") as mm_ps:
                for n in range(NCH):
                    xk = xtiles[n]
                    for m in range(2):
                        a_ps = mm_ps.tile([P, CH], f32, tag="A", name=f"A{n}{m}")
                        b_ps = mm_ps.tile([P, CH], f32, tag="B", name=f"B{n}{m}")
                        for ps_t, eT in ((a_ps, eaT), (b_ps, ebT)):
                            for k in range(2):
                                for s in range(CH // 512):
                                    sl = slice(s * 512, (s + 1) * 512)
                                    nc.tensor.matmul(ps_t[:, sl],
                                                     eT[k][:, m * P:(m + 1) * P],
                                                     xk[:, k, sl],
                                                     start=(k == 0), stop=(k == 1))
                        # out = (A + u) * (cAB*B + cA) + w
                        s_sb = ep.tile([P, CH], f32, tag="s", name=f"s{n}{m}")
                        nc.scalar.activation(out=s_sb[:], in_=b_ps[:], func=Act.Identity,
                                             scale=cAB2[:, m:m + 1], bias=cA2[:, m:m + 1])
                        p_sb = ep.tile([P, CH], f32, tag="p", name=f"p{n}{m}")
                        nc.vector.scalar_tensor_tensor(out=p_sb[:], in0=a_ps[:],
                                                       scalar=cU2[:, m:m + 1], in1=s_sb[:],
                                                       op0=Alu.add, op1=Alu.mult)
                        o_sb = ep.tile([P, CH], f32, tag="o", name=f"o{n}{m}")
                        nc.vector.tensor_scalar_add(o_sb[:], p_sb[:], cW2[:, m:m + 1])
                        nc.sync.dma_start(out=out_d[m * P:(m + 1) * P, n * CH:(n + 1) * CH],
                                          in_=o_sb[:])

    nc.compile()
    return nc


def _get_nc():
    if "nc" not in _CACHE:
        _CACHE["nc"] = _build_bass()
    return _CACHE["nc"]


def _make_blob(wt: np.ndarray) -> np.ndarray:
    blob = np.zeros((P, BLOB_W), dtype=np.float32)
    blob[:, 0:128] = np.eye(P, dtype=np.float32)
    blob[:, 128] = 1.0
    blob[:16, 129:134] = _sign_matrix()
    blob[:16, 134:390] = wt
    return blob


def _run(x, wa, wb, wt, trace=False, **spmd_kwargs):
    from concourse import bass_utils

    nc = _get_nc()
    x = np.ascontiguousarray(np.asarray(x, dtype=np.float32))
    wa = np.ascontiguousarray(np.asarray(wa, dtype=np.float32))
    wb = np.ascontiguousarray(np.asarray(wb, dtype=np.float32))
    wt = np.ascontiguousarray(np.asarray(wt, dtype=np.float32))
    blob = _make_blob(wt)

    in_maps = []
    for c in range(N_CORES):
        in_maps.append({
            "xs": np.ascontiguousarray(x[:, c * BSH:(c + 1) * BSH]),
            "wa": wa, "wb": wb, "blob": blob,
        })
    res = bass_utils.run_bass_kernel_spmd(nc, in_maps, core_ids=list(range(N_CORES)),
                                          trace=trace, **spmd_kwargs)
    out = np.concatenate([res.results[c]["out"] for c in range(N_CORES)], axis=1)
    return out, res


def kernel(x, wa, wb, wt):
    out, _ = _run(x, wa, wb, wt, trace=False)
    return out


# revision 8
# speedup vs baseline: 1.4053x; 1.0623x over previous
"""Trainium2 Bass kernel for the fused soft-logic-gate layer.

Reference computation:
    pa = softmax(wa, axis=1); pb = softmax(wb, axis=1); pt = softmax(wt, axis=0)
    A = pa @ x; B = pb @ x
    out = sum_g pt[g,:,None] * gate_g(A, B)        (16 soft logic gates)

Every gate is affine in {1, A, B, A*B}, so the 16-gate table collapses to
    out = c0 + cA*A + cB*B + cAB*(A*B)
with four per-row coefficient vectors derived from pt.  Folding the softmax
denominators of wa/wb into those coefficients lets the matmuls run on the raw
exp() weights, and factoring
    out = (A + u) * (cAB*B + cA) + w,   u = cB/cAB,  w = c0 - cA*u
leaves one ACT op + two DVE ops per tile.  The device work is two
[256,256]x[256,b] float32r (TF32) matmuls plus that elementwise pass —
memory-bound on streaming x in and out once.

Sharding: batch axis of x split evenly across 8 NeuronCores (data parallel),
weights replicated.
"""

import os
import sys

for _p in ("/opt/trn_rl_repo",):
    if _p not in sys.path and os.path.isdir(_p):
        sys.path.insert(0, _p)

import numpy as np

SIZE = 256
PREV = 256
BATCH = 32768
N_CORES = 8
BSH = BATCH // N_CORES  # per-core batch shard
CH = 1024               # epilogue chunk width (2 PSUM banks)
NCH = BSH // CH
P = 128

# constants blob layout (f32, [128, 390]):
#   [:, 0:128]    identity
#   [:, 128]      ones column
#   [:16, 129:134] sign matrix [16, 5] (cols: sum, c0, cA, cB, cAB)
#   [:16, 134:390] wt [16, 256]
BLOB_W = 390

_CACHE = {}


def _sign_matrix() -> np.ndarray:
    """[16,5] f32 columns: [colsum, c0, cA, cB, cAB] — gate-table
    coefficients of {1, A, B, A*B} preceded by the softmax denominator."""
    S = np.zeros((16, 5), dtype=np.float32)
    S[:, 0] = 1.0
    S[8:16, 1] = 1.0
    for g in (2, 3, 6, 7):
        S[g, 2] += 1.0
    for g in (8, 9, 12, 13):
        S[g, 2] -= 1.0
    for g in (4, 5, 6, 7):
        S[g, 3] += 1.0
    for g in (8, 9, 10, 11):
        S[g, 3] -= 1.0
    for g, v in {1: 1, 2: -1, 4: -1, 6: -2, 7: -1, 8: 1, 9: 2, 11: 1, 13: 1, 14: -1}.items():
        S[g, 4] = v
    return S


def _build_bass():
    import concourse.bacc as bacc
    import concourse.tile as tile
    import concourse.mybir as mybir

    f32 = mybir.dt.float32
    f32r = mybir.dt.float32r
    Act = mybir.ActivationFunctionType
    Alu = mybir.AluOpType

    nc = bacc.Bacc(trn_type="TRN2", target_bir_lowering=False, debug=False,
                   num_devices=N_CORES)

    xs_d = nc.dram_tensor("xs", [PREV, BSH], f32r, kind="ExternalInput").ap()
    wa_d = nc.dram_tensor("wa", [SIZE, PREV], f32, kind="ExternalInput").ap()
    wb_d = nc.dram_tensor("wb", [SIZE, PREV], f32, kind="ExternalInput").ap()
    bl_d = nc.dram_tensor("blob", [P, BLOB_W], f32, kind="ExternalInput").ap()
    out_d = nc.dram_tensor("out", [SIZE, BSH], f32, kind="ExternalOutput").ap()

    # [p, k, b] views for single-DMA transfers
    xs_v = xs_d.rearrange("(k p) b -> p k b", p=P)
    wa_v = wa_d.rearrange("(m p) c -> p m c", p=P)
    wb_v = wb_d.rearrange("(m p) c -> p m c", p=P)

    with tile.TileContext(nc) as tc:
        with tc.tile_pool(name="consts", bufs=1) as consts, \
             tc.tile_pool(name="weights", bufs=1) as weights, \
             tc.tile_pool(name="coefs", bufs=1) as coefs, \
             tc.tile_pool(name="xp", bufs=4) as xp:

            blob = consts.tile([P, BLOB_W], f32)
            nc.sync.dma_start(out=blob[:], in_=bl_d[:])
            ident = blob[:, 0:128]
            onesc = blob[:, 128:129]
            smat = blob[:16, 129:134]
            wts = blob[:16, 134:390]

            # tiny early Exp forces the ACT table load off the critical path
            dummy = consts.tile([1, 1], f32)
            nc.scalar.activation(out=dummy[:], in_=blob[0:1, 128:129], func=Act.Exp)

            wa_sb = consts.tile([P, 2, PREV], f32)
            nc.sync.dma_start(out=wa_sb[:], in_=wa_v[:])
            wb_sb = consts.tile([P, 2, PREV], f32)
            nc.sync.dma_start(out=wb_sb[:], in_=wb_v[:])

            # prefetch all x chunks
            xtiles = []
            for n in range(NCH):
                xt = xp.tile([P, 2, CH], f32r, tag="x", name=f"x{n}")
                nc.sync.dma_start(out=xt[:], in_=xs_v[:, :, n * CH:(n + 1) * CH])
                xtiles.append(xt)

            # Transposed exp(weights), float32r, live for the whole kernel:
            # eaT[p] is [128(prev-block p), 256(size)].
            eaT = [weights.tile([P, SIZE], f32r, tag=f"eaT{p}", name=f"eaT{p}") for p in range(2)]
            ebT = [weights.tile([P, SIZE], f32r, tag=f"ebT{p}", name=f"ebT{p}") for p in range(2)]

            # [128,2] coefficient tiles (m as free dim):
            cA2 = coefs.tile([P, 2], f32, tag="cA2")
            cAB2 = coefs.tile([P, 2], f32, tag="cAB2")
            cU2 = coefs.tile([P, 2], f32, tag="cU2")
            cW2 = coefs.tile([P, 2], f32, tag="cW2")

            # ---- weight preprocessing ----
            with tc.tile_pool(name="prep", bufs=2) as prep, \
                 tc.tile_pool(name="prep_ps", bufs=3, space="PSUM") as prep_ps, \
                 tc.tile_pool(name="coef_ps", bufs=1, space="PSUM") as coef_ps:

                # pt-coefficient path first: it only needs the blob
                ept = prep.tile([16, SIZE], f32, tag="ept")
                nc.scalar.activation(out=ept[:], in_=wts, func=Act.Exp)
                cps = coef_ps.tile([P, 10], f32, tag="cps")
                for m in range(2):
                    nc.tensor.matmul(cps[:, m * 5:(m + 1) * 5],
                                     ept[:, m * P:(m + 1) * P], smat,
                                     start=True, stop=True)
                cpsv = cps[:].rearrange("p (m c) -> p c m", m=2)
                rpt2 = prep.tile([P, 2], f32, tag="rpt2")
                nc.vector.reciprocal(out=rpt2[:], in_=cpsv[:, 0, :])
                rcabn = prep.tile([P, 2], f32, tag="rcabn")
                nc.vector.reciprocal(out=rcabn[:], in_=cpsv[:, 4, :])

                # exp in natural layout (one wide ACT op per weight), row sums
                # on DVE, then PE-transpose each block and copy out as float32r
                rsa = prep.tile([P, 2], f32, tag="rsa")
                rsb = prep.tile([P, 2], f32, tag="rsb")
                for w_sb, eT, rs, nm in ((wa_sb, eaT, rsa, "a"), (wb_sb, ebT, rsb, "b")):
                    e_nat = prep.tile([P, 2, PREV], f32, tag=f"e{nm}", name=f"e{nm}")
                    nc.scalar.activation(out=e_nat[:], in_=w_sb[:], func=Act.Exp)
                    for m in range(2):
                        nc.vector.tensor_reduce(out=rs[:, m:m + 1], in_=e_nat[:, m, :],
                                                axis=mybir.AxisListType.X, op=Alu.add)
                    for m in range(2):
                        for p in range(2):
                            tp = prep_ps.tile([P, P], f32, tag="tps", name=f"tp{nm}{m}{p}")
                            nc.tensor.transpose(tp[:], e_nat[:, m, p * P:(p + 1) * P], ident)
                            nc.scalar.copy(out=eT[p][:, m * P:(m + 1) * P], in_=tp[:])

                ra2 = prep.tile([P, 2], f32, tag="ra2")
                nc.vector.reciprocal(out=ra2[:], in_=rsa[:])
                rb2 = prep.tile([P, 2], f32, tag="rb2")
                nc.vector.reciprocal(out=rb2[:], in_=rsb[:])

                h2 = prep.tile([P, 2], f32, tag="h2")
                nc.vector.tensor_tensor(out=h2[:], in0=rpt2[:], in1=ra2[:], op=Alu.mult)
                nc.vector.tensor_tensor(out=cA2[:], in0=cpsv[:, 2, :], in1=h2[:], op=Alu.mult)
                g2 = prep.tile([P, 2], f32, tag="g2")
                nc.vector.tensor_tensor(out=g2[:], in0=h2[:], in1=rb2[:], op=Alu.mult)
                nc.vector.tensor_tensor(out=cAB2[:], in0=cpsv[:, 4, :], in1=g2[:], op=Alu.mult)

                # u = cBn * sa / cABn ;  w = c0n*rpt - cA*u
                u2a = prep.tile([P, 2], f32, tag="u2a")
                nc.vector.tensor_tensor(out=u2a[:], in0=cpsv[:, 3, :], in1=rcabn[:], op=Alu.mult)
                nc.vector.tensor_tensor(out=cU2[:], in0=u2a[:], in1=rsa[:], op=Alu.mult)
                c02 = prep.tile([P, 2], f32, tag="c02")
                nc.vector.tensor_tensor(out=c02[:], in0=cpsv[:, 1, :], in1=rpt2[:], op=Alu.mult)
                t2 = prep.tile([P, 2], f32, tag="t2")
                nc.vector.tensor_tensor(out=t2[:], in0=cA2[:], in1=cU2[:], op=Alu.mult)
                nc.vector.tensor_tensor(out=cW2[:], in0=c02[:], in1=t2[:], op=Alu.subtract)

            # ---- main loop ----
            with tc.tile_pool(name="ep", bufs=3) as ep, \
                 tc.tile_pool(name="mm_ps", bufs=2, space="PSUM") as mm_ps:
                for n in range(NCH):
                    xk = xtiles[n]
                    for m in range(2):
                        a_ps = mm_ps.tile([P, CH], f32, tag="A", name=f"A{n}{m}")
                        b_ps = mm_ps.tile([P, CH], f32, tag="B", name=f"B{n}{m}")
                        for ps_t, eT in ((a_ps, eaT), (b_ps, ebT)):
                            for k in range(2):
                                for s in range(CH // 512):
                                    sl = slice(s * 512, (s + 1) * 512)
                                    nc.tensor.matmul(ps_t[:, sl],
                                                     eT[k][:, m * P:(m + 1) * P],
                                                     xk[:, k, sl],
                                                     start=(k == 0), stop=(k == 1))
                        # out = (A + u) * (cAB*B + cA) + w
                        s_sb = ep.tile([P, CH], f32, tag="s", name=f"s{n}{m}")
                        nc.scalar.activation(out=s_sb[:], in_=b_ps[:], func=Act.Identity,
                                             scale=cAB2[:, m:m + 1], bias=cA2[:, m:m + 1])
                        p_sb = ep.tile([P, CH], f32, tag="p", name=f"p{n}{m}")
                        nc.vector.scalar_tensor_tensor(out=p_sb[:], in0=a_ps[:],
                                                       scalar=cU2[:, m:m + 1], in1=s_sb[:],
                                                       op0=Alu.add, op1=Alu.mult)
                        o_sb = ep.tile([P, CH], f32, tag="o", name=f"o{n}{m}")
                        nc.vector.tensor_scalar_add(o_sb[:], p_sb[:], cW2[:, m:m + 1])
                        if n == NCH - 1 and m == 1:
                            for h in range(2):
                                hw = CH // 2
                                nc.sync.dma_start(
                                    out=out_d[m * P:(m + 1) * P,
                                              n * CH + h * hw:n * CH + (h + 1) * hw],
                                    in_=o_sb[:, h * hw:(h + 1) * hw])
                        else:
                            nc.sync.dma_start(out=out_d[m * P:(m + 1) * P, n * CH:(n + 1) * CH],
                                              in_=o_sb[:])

    nc.compile()
    return nc


def _get_nc():
    if "nc" not in _CACHE:
        _CACHE["nc"] = _build_bass()
    return _CACHE["nc"]


def _make_blob(wt: np.ndarray) -> np.ndarray:
    blob = np.zeros((P, BLOB_W), dtype=np.float32)
    blob[:, 0:128] = np.eye(P, dtype=np.float32)
    blob[:, 128] = 1.0
    blob[:16, 129:134] = _sign_matrix()
    blob[:16, 134:390] = wt
    return blob


def _run(x, wa, wb, wt, trace=False, **spmd_kwargs):
    from concourse import bass_utils

    nc = _get_nc()
    x = np.ascontiguousarray(np.asarray(x, dtype=np.float32))
    wa = np.ascontiguousarray(np.asarray(wa, dtype=np.float32))
    wb = np.ascontiguousarray(np.asarray(wb, dtype=np.float32))
    wt = np.ascontiguousarray(np.asarray(wt, dtype=np.float32))
    blob = _make_blob(wt)

    in_maps = []
    for c in range(N_CORES):
        in_maps.append({
            "xs": np.ascontiguousarray(x[:, c * BSH:(c + 1) * BSH]),
            "wa": wa, "wb": wb, "blob": blob,
        })
    res = bass_utils.run_bass_kernel_spmd(nc, in_maps, core_ids=list(range(N_CORES)),
                                          trace=trace, **spmd_kwargs)
    out = np.concatenate([res.results[c]["out"] for c in range(N_CORES)], axis=1)
    return out, res


def kernel(x, wa, wb, wt):
    out, _ = _run(x, wa, wb, wt, trace=False)
    return out
